# revision 5
# baseline (speedup 1.0000x reference)
"""Expert-parallel MoE MLP kernel for TRN2 (8 NeuronCores).

Reference computation (all experts, dense routing):
    hidden = einsum("bnd,edh->benh", x, w1); hidden = gelu(hidden)
    out    = einsum("benh,ehd->bnde", hidden, w2)        # [b, n, d4, e]

Sharding: expert-parallel, 2 experts per core (16 experts / 8 cores); x is
replicated. Each core computes, for its experts e:
    hT[e] = gelu(W1[e].T @ X.T)        # [h, tok] layout, h on partitions
    outT[e] = W2[e].T @ hT[e]          # [d4, tok] layout
which keeps the contraction dim on SBUF partitions for both matmuls with no
on-device transposes: W1 (d,h) / W2 (h,d4) load in natural layout as lhsT, and
X.T is prepared once on the host. All matmul operands are bf16 (hidden is
written back from PSUM as bf16 by the gelu activation): the PE runs at the
same 1 row/cycle as fp32r but with FWL weight loads fully hidden, and input
DMA traffic halves. PSUM accumulation stays fp32; end-to-end rel err ~3e-3.

DMA issue is spread across queues so the startup chain is not serialized:
weights go out on the gpsimd queue while x streams on the sync queue, and
output DMAs ride the vector queue (ordered after their producing copy anyway).
The last token tile's second matmul is split into two half-width PSUM groups
so the final copy+DMA tail is half as long.

The [e, d4, tok] device layout is re-interleaved to [b, n, d4, e] on the host.
"""

import sys

import numpy as np

for _p in ("/opt/trn_rl_repo", "/root/.axon_site/_ro/trn_rl_repo"):
    if _p not in sys.path:
        sys.path.append(_p)

import ml_dtypes

import concourse.bacc as bacc
import concourse.mybir as mybir
import concourse.tile as tile
from concourse.bass_utils import run_bass_kernel_spmd

F32 = mybir.dt.float32
BF16 = mybir.dt.bfloat16
NP_BF16 = ml_dtypes.bfloat16

N_CORES = 8
E = 16                 # total experts
E_LOC = E // N_CORES   # experts per core
D = 512                # model dim (contraction of mm1)
H = 512                # hidden dim (contraction of mm2)
D4 = 128               # output dim per expert
NTOK = 4 * 2048        # tokens
TT = 512               # token tile (matmul moving free dim)
P = 128
N_T = NTOK // TT


def _build_program():
    nc = bacc.Bacc("TRN2", target_bir_lowering=False, debug=False)
    xT = nc.declare_dram_parameter("xT", [D, NTOK], BF16, isOutput=False)
    w1 = nc.declare_dram_parameter("w1", [E_LOC, D, H], BF16, isOutput=False)
    w2 = nc.declare_dram_parameter("w2", [E_LOC, H, D4], BF16, isOutput=False)
    outT = nc.declare_dram_parameter("outT", [E_LOC, D4, NTOK], F32, isOutput=True)

    gelu = mybir.ActivationFunctionType.Gelu
    n_dt = D // P   # 4 k-tiles of mm1
    n_ht = H // P   # 4 k-tiles of mm2

    with tile.TileContext(nc) as tc:
        with (
            tc.tile_pool(name="wpool", bufs=1) as wpool,
            tc.tile_pool(name="xpool", bufs=4) as xpool,
            tc.tile_pool(name="hpool", bufs=2) as hpool,
            tc.tile_pool(name="opool", bufs=4) as opool,
            tc.tile_pool(name="ps1p", bufs=4, space="PSUM") as ps1p,
            tc.tile_pool(name="ps2p", bufs=4, space="PSUM") as ps2p,
        ):
            # Weights resident in SBUF for the whole kernel, natural layout.
            w1_sb = wpool.tile([P, E_LOC, n_dt, H], BF16, name="w1_sb", tag="w1")
            w1_r = w1.rearrange("e (dt p) h -> p e dt h", p=P)
            w2_sb = wpool.tile([P, E_LOC, n_ht, D4], BF16, name="w2_sb", tag="w2")
            w2_r = w2.rearrange("e (ht p) d -> p e ht d", p=P)
            xT_r = xT.rearrange("(dt p) n -> p dt n", p=P)

            x_tiles = {}

            def load_x(t):
                tok = slice(t * TT, (t + 1) * TT)
                x_sb = xpool.tile([P, n_dt, TT], BF16, name="x_sb", tag="x")
                nc.sync.dma_start(x_sb, xT_r[:, :, tok])
                x_tiles[t] = x_sb

            # Startup: x0 piecewise on the sync queue (the first matmul only
            # needs the 128KB dt0 slice), weights concurrently on the gpsimd
            # queue — w1[e0] also piecewise to track the matmul k-loop.
            tok0 = slice(0, TT)
            x0_sb = xpool.tile([P, n_dt, TT], BF16, name="x_sb", tag="x")
            nc.sync.dma_start(x0_sb[:, 0], xT_r[:, 0, tok0])
            for dt_i in range(n_dt):
                nc.gpsimd.dma_start(w1_sb[:, 0, dt_i], w1_r[:, 0, dt_i])
            for dt_i in range(1, n_dt):
                nc.sync.dma_start(x0_sb[:, dt_i], xT_r[:, dt_i, tok0])
            x_tiles[0] = x0_sb
            for e in range(1, E_LOC):
                nc.gpsimd.dma_start(w1_sb[:, e], w1_r[:, e])
            for e in range(E_LOC):
                nc.gpsimd.dma_start(w2_sb[:, e], w2_r[:, e])

            def mm2_store(e, hT_sb, t, width):
                """Second matmul + copy-out + DMA for expert e, in `width`-wide
                token chunks of tile t."""
                for c in range(TT // width):
                    sl = slice(c * width, (c + 1) * width)
                    ps2 = ps2p.tile([P, width], F32, name="ps2", tag="ps2")
                    for ht in range(n_ht):
                        nc.tensor.matmul(
                            ps2,
                            w2_sb[:, e, ht, :],
                            hT_sb[:, ht, sl],
                            start=(ht == 0),
                            stop=(ht == n_ht - 1),
                        )
                    o_sb = opool.tile([P, width], F32, name="o_sb", tag="o")
                    nc.vector.tensor_copy(o_sb, ps2)
                    nc.gpsimd.dma_start(
                        outT[e, :, t * TT + c * width : t * TT + (c + 1) * width],
                        o_sb,
                    )

            for t in range(N_T):
                if t + 1 < N_T and t + 1 not in x_tiles:
                    load_x(t + 1)  # prefetch next tile one iteration ahead
                x_sb = x_tiles.pop(t)
                hT_tiles = []
                for e in range(E_LOC):
                    hT_sb = hpool.tile([P, n_ht, TT], BF16, name="hT_sb", tag="h")
                    for ht in range(n_ht):
                        ps1 = ps1p.tile([P, TT], F32, name="ps1", tag="ps1")
                        for dt_i in range(n_dt):
                            nc.tensor.matmul(
                                ps1,
                                w1_sb[:, e, dt_i, ht * P : (ht + 1) * P],
                                x_sb[:, dt_i],
                                start=(dt_i == 0),
                                stop=(dt_i == n_dt - 1),
                            )
                        nc.scalar.activation(hT_sb[:, ht, :], ps1, gelu)
                    hT_tiles.append(hT_sb)
                # Final tile: half-width chunks so the tail copy+DMA is short.
                width = TT // 2 if t == N_T - 1 else TT
                for e in range(E_LOC):
                    mm2_store(e, hT_tiles[e], t, width)

    nc.finalize()
    return nc


_NC = None


def _get_program():
    global _NC
    if _NC is None:
        _NC = _build_program()
    return _NC


def _in_maps(x: np.ndarray, w1: np.ndarray, w2: np.ndarray):
    xT = np.ascontiguousarray(x.reshape(NTOK, D).T).astype(NP_BF16)
    w1b = w1.astype(NP_BF16)
    w2b = w2.astype(NP_BF16)
    return [
        {
            "xT": xT,
            "w1": np.ascontiguousarray(w1b[c * E_LOC : (c + 1) * E_LOC]),
            "w2": np.ascontiguousarray(w2b[c * E_LOC : (c + 1) * E_LOC]),
        }
        for c in range(N_CORES)
    ]


def kernel(x: np.ndarray, w1: np.ndarray, w2: np.ndarray, **_) -> np.ndarray:
    """Full inputs in, full output out; expert-parallel across 8 NeuronCores."""
    nc = _get_program()
    res = run_bass_kernel_spmd(nc, _in_maps(x, w1, w2), list(range(N_CORES)))

    full = np.stack([res.results[c]["outT"] for c in range(N_CORES)], axis=0)
    full = full.reshape(E, D4, NTOK)              # [e, d4, tok]
    out = full.transpose(2, 1, 0)                 # [tok, d4, e]
    return np.ascontiguousarray(out.reshape(4, 2048, D4, E), dtype=np.float32)
